# revision 9
# baseline (speedup 1.0000x reference)
"""Distributed Trainium2 kernel for the ADMM-NN fixed-point iteration:

    for _ in range(N):
        x = W @ x + b
        x[idx1:idx2] = clip(x[idx1:idx2], l, u)

Strategy (8 NeuronCores, tensor-parallel):
  - Row-shard W: core i owns 1024 rows.  The rows are PERMUTED host-side so
    that every core owns 768 un-clamped rows + 256 clamped rows; the clamp
    is then the same local slice y[768:1024] on every core (SPMD-uniform).
  - W is stored bf16, resident in SBUF (16 MB/core) -> after the initial
    load there is no HBM weight traffic at all; each iteration is a pure
    TensorEngine GEMV (x-stationary: lhsT = x k-tile [128,1], rhs = W^T
    streaming at full rate) + small AllGathers.
  - Software pipeline: y is produced in C chunks (one PSUM bank each).
    Each chunk is all-gathered (in bf16) while the TensorEngine computes
    the remaining chunks; the x-layout gamma is arranged so that the
    k-tiles of the next iteration that depend on chunk c's gather are
    exactly the ones consumed while chunk c's gather is still covered by
    C-1 phases of compute.
  - The wire format is bf16: y chunks are produced directly in bf16 by the
    DVE epilogue, gathered in bf16, and DMA'd straight into the matmul
    operand tiles -- no cast on the critical path.  The final iteration
    skips the gather and writes f32.

kernel(**inputs) takes the FULL unsharded inputs and returns the FULL
output, matching reference.reference().
"""

import numpy as np
import ml_dtypes

NCORES = 8
D = 8192
ROWS = D // NCORES  # 1024 rows per core
NT = 64  # contraction k-tiles of 128
P = 128  # partitions
NCHUNKS = 2  # y chunks per iteration (one PSUM bank each)
CW = ROWS // NCHUNKS  # chunk width in outputs
CT = NT // NCHUNKS  # k-tiles per chunk

_nc_cache = {}


def _perm(idx1, idx2):
    """Permuted order: core i owns un-clamped rows [un*i, un*(i+1)) followed
    by clamped rows [idx1 + seg*i, idx1 + seg*(i+1))."""
    assert idx2 == D and idx1 % NCORES == 0
    seg = (idx2 - idx1) // NCORES
    un = ROWS - seg
    assert un * NCORES == idx1
    parts = []
    for i in range(NCORES):
        parts.append(np.arange(un * i, un * (i + 1)))
        parts.append(np.arange(idx1 + seg * i, idx1 + seg * (i + 1)))
    return np.concatenate(parts), un


def _gamma():
    """x_sb[p, t] holds x'[gamma[p, t]].  Chunk c (t in [16c,16c+16)) covers
    each core's y[256c:256c+256] in AllGather-output flat order."""
    p = np.arange(P)[:, None]
    t = np.arange(NT)[None, :]
    return (p // 16) * 1024 + (t // CT) * CW + (p % 16) * CT + (t % CT)


def _build_nc(n_iter, clamp_lo, l_val, u_val):
    import concourse.bacc as bacc
    import concourse.mybir as mybir
    from concourse import tile
    from concourse.bass import _add_dep_helper

    # clamp region must sit inside the last chunk
    cl = clamp_lo - (NCHUNKS - 1) * CW
    assert 0 <= cl < CW
    nc = bacc.Bacc(None, target_bir_lowering=False, num_devices=NCORES)
    NWCH = 16  # W arrives as 16 chunks of 4 k-tiles each
    w_ext = [
        nc.declare_dram_parameter(
            f"W{c}", [P, (NT // NWCH) * ROWS], mybir.dt.bfloat16, isOutput=False
        )
        for c in range(NWCH)
    ]
    x0_ext = nc.declare_dram_parameter("x0", [P, NT], mybir.dt.bfloat16, isOutput=False)
    b_ext = nc.declare_dram_parameter("bias", [1, ROWS], mybir.dt.float32, isOutput=False)
    out_ext = nc.declare_dram_parameter("out", [1, ROWS], mybir.dt.float32, isOutput=True)

    with tile.TileContext(nc) as tc:
        with (
            tc.tile_pool(name="wpool", bufs=1) as wpool,
            tc.tile_pool(name="cpool", bufs=1) as cpool,
            tc.tile_pool(name="xpool", bufs=2) as xpool,
            tc.tile_pool(name="ypool", bufs=2) as ypool,
            tc.tile_pool(name="ps", bufs=2, space="PSUM") as pspool,
            tc.tile_pool(name="dram", bufs=2, space="DRAM") as dpool,
        ):
            wt = []
            for c in range(NWCH):
                w = wpool.tile([P, (NT // NWCH) * ROWS], mybir.dt.bfloat16, tag=f"W{c}")
                nc.sync.dma_start(w[:], w_ext[c][:])
                wt.append(w)
            b_sb = cpool.tile([1, ROWS], mybir.dt.float32, tag="b")
            nc.sync.dma_start(b_sb[:], b_ext[:])

            xb = []
            for c in range(NCHUNKS):
                x = xpool.tile([P, CT], mybir.dt.bfloat16, tag=f"xb{c}")
                nc.sync.dma_start(x[:], x0_ext[:, c * CT : (c + 1) * CT])
                xb.append(x)

            prev_mm = None  # last matmul of the previous phase: enforce strict
            # PE-stream phase order so each chunk's accumulation finishes (and
            # its AllGather fires) as early as possible.
            for k in range(n_iter):
                last = k == n_iter - 1
                xb_next = list(xb)
                for c in range(NCHUNKS):
                    ps = pspool.tile([1, CW], mybir.dt.float32, tag=f"ps{c}")
                    for t in range(NT):
                        wc, r = t // 4, t % 4
                        mm = nc.tensor.matmul(
                            ps[:, :],
                            xb[t // CT][:, t % CT : t % CT + 1],
                            wt[wc][:, r * ROWS + c * CW : r * ROWS + (c + 1) * CW],
                            start=(t == 0),
                            stop=(t == NT - 1),
                        )
                        if t == 0 and prev_mm is not None:
                            _add_dep_helper(
                                mm.ins,
                                prev_mm.ins,
                                sync=False,
                                reason="strict phase order on PE",
                            )
                    prev_mm = mm
                    is_clamp = c == NCHUNKS - 1
                    ydt = mybir.dt.float32 if last else mybir.dt.bfloat16
                    y = ypool.tile([1, CW], ydt, tag=f"y{c}_{'f' if last else 'b'}")
                    nc.vector.tensor_tensor(
                        y[:, :],
                        ps[:, :],
                        b_sb[:, c * CW : (c + 1) * CW],
                        op=mybir.AluOpType.add,
                    )
                    if is_clamp:
                        nc.vector.tensor_scalar(
                            y[:, cl:CW],
                            y[:, cl:CW],
                            float(l_val),
                            float(u_val),
                            mybir.AluOpType.max,
                            mybir.AluOpType.min,
                        )
                    if last:
                        nc.sync.dma_start(out_ext[:, c * CW : (c + 1) * CW], y[:])
                    else:
                        agin = dpool.tile([1, CW], mybir.dt.bfloat16, tag=f"agin{c}")
                        agout = dpool.tile([P, CT], mybir.dt.bfloat16, tag=f"agout{c}")
                        nc.sync.dma_start(agin[:], y[:])
                        nc.gpsimd.collective_compute(
                            "AllGather",
                            mybir.AluOpType.bypass,
                            replica_groups=[list(range(NCORES))],
                            ins=[agin.opt()],
                            outs=[agout.opt()],
                        )
                        xn = xpool.tile([P, CT], mybir.dt.bfloat16, tag=f"xb{c}")
                        nc.scalar.dma_start(xn[:], agout[:])
                        xb_next[c] = xn
                xb = xb_next
    nc.compile()
    return nc


def _get_nc(n_iter, clamp_lo, l_val, u_val):
    key = (n_iter, clamp_lo, float(l_val), float(u_val))
    if key not in _nc_cache:
        _nc_cache[key] = _build_nc(n_iter, clamp_lo, l_val, u_val)
    return _nc_cache[key]


def _prep_in_maps(x, W, b, idx1, idx2):
    perm, _un = _perm(idx1, idx2)
    g = _gamma()
    colidx = perm[g]  # [128, 64] original column index per (p, t)
    xp = np.asarray(x, np.float32)[perm]
    bp = np.asarray(b, np.float32)[perm]
    bf16 = ml_dtypes.bfloat16
    x0_layout = np.ascontiguousarray(xp[g]).astype(bf16)
    NWCH = 16
    in_maps = []
    for i in range(NCORES):
        rows_i = perm[ROWS * i : ROWS * (i + 1)]
        Wi = W[rows_i]  # [1024, 8192]
        Wc = Wi[:, colidx.reshape(-1)].reshape(ROWS, P, NT)  # [n, p, t]
        Wt = np.ascontiguousarray(
            np.transpose(Wc, (1, 2, 0)).reshape(P, NT * ROWS)
        ).astype(bf16)  # Wt[p, t*1024 + n]
        m = {
            f"W{c}": np.ascontiguousarray(
                Wt[:, c * (NT // NWCH) * ROWS : (c + 1) * (NT // NWCH) * ROWS]
            )
            for c in range(NWCH)
        }
        m["x0"] = x0_layout
        m["bias"] = np.ascontiguousarray(bp[ROWS * i : ROWS * (i + 1)].reshape(1, ROWS))
        in_maps.append(m)
    return in_maps, perm


def run(x, W, b, l, u, idx1, idx2, N, trace=False, trace_kwargs=None):
    from concourse.bass_utils import run_bass_kernel_spmd

    x = np.asarray(x, np.float32)
    W = np.asarray(W, np.float32)
    b = np.asarray(b, np.float32)
    l = float(np.asarray(l))
    u = float(np.asarray(u))
    idx1 = int(np.asarray(idx1))
    idx2 = int(np.asarray(idx2))
    N = int(np.asarray(N))
    assert x.shape == (D,) and W.shape == (D, D) and b.shape == (D,)
    assert N >= 1

    seg = (idx2 - idx1) // NCORES
    clamp_lo = ROWS - seg
    nc = _get_nc(N, clamp_lo, l, u)
    in_maps, perm = _prep_in_maps(x, W, b, idx1, idx2)
    res = run_bass_kernel_spmd(
        nc,
        in_maps,
        core_ids=list(range(NCORES)),
        trace=trace,
        **(trace_kwargs or {}),
    )
    chunks = [np.asarray(res.results[i]["out"], np.float32).reshape(ROWS) for i in range(NCORES)]
    xp_final = np.concatenate(chunks)
    out = np.empty(D, np.float32)
    out[perm] = xp_final
    return out, res


def kernel(**inputs):
    out, _ = run(
        inputs["x"],
        inputs["W"],
        inputs["b"],
        inputs["l"],
        inputs["u"],
        inputs["idx1"],
        inputs["idx2"],
        inputs["N"],
        trace=False,
    )
    return out


# revision 12
# speedup vs baseline: 1.0490x; 1.0490x over previous
"""Distributed Trainium2 kernel for the ADMM-NN fixed-point iteration:

    for _ in range(N):
        x = W @ x + b
        x[idx1:idx2] = clip(x[idx1:idx2], l, u)

Strategy (8 NeuronCores, tensor-parallel):
  - Row-shard W: core i owns 1024 rows.  The rows are PERMUTED host-side so
    that every core owns 768 un-clamped rows + 256 clamped rows; the clamp
    is then the same local slice y[768:1024] on every core (SPMD-uniform).
  - W is stored bf16, resident in SBUF (16 MB/core) -> after the initial
    load there is no HBM weight traffic at all; each iteration is a pure
    TensorEngine GEMV (x-stationary: lhsT = x k-tile [128,1], rhs = W^T
    streaming at full rate) + small AllGathers.
  - Software pipeline: y is produced in C chunks (one PSUM bank each).
    Each chunk is all-gathered (in bf16) while the TensorEngine computes
    the remaining chunks; the x-layout gamma is arranged so that the
    k-tiles of the next iteration that depend on chunk c's gather are
    exactly the ones consumed while chunk c's gather is still covered by
    C-1 phases of compute.
  - The wire format is bf16: y chunks are produced directly in bf16 by the
    DVE epilogue, gathered in bf16, and DMA'd straight into the matmul
    operand tiles -- no cast on the critical path.  The final iteration
    skips the gather and writes f32.

kernel(**inputs) takes the FULL unsharded inputs and returns the FULL
output, matching reference.reference().
"""

import numpy as np
import ml_dtypes

NCORES = 8
D = 8192
ROWS = D // NCORES  # 1024 rows per core
NT = 64  # contraction k-tiles of 128
P = 128  # partitions
NCHUNKS = 2  # y chunks per iteration (one PSUM bank each)
CW = ROWS // NCHUNKS  # chunk width in outputs
CT = NT // NCHUNKS  # k-tiles per chunk
PAIR = 24  # k-tiles whose psA/psB matmuls are paired (shared LDWEIGHTS)

_nc_cache = {}


def _perm(idx1, idx2):
    """Permuted order: core i owns un-clamped rows [un*i, un*(i+1)) followed
    by clamped rows [idx1 + seg*i, idx1 + seg*(i+1))."""
    assert idx2 == D and idx1 % NCORES == 0
    seg = (idx2 - idx1) // NCORES
    un = ROWS - seg
    assert un * NCORES == idx1
    parts = []
    for i in range(NCORES):
        parts.append(np.arange(un * i, un * (i + 1)))
        parts.append(np.arange(idx1 + seg * i, idx1 + seg * (i + 1)))
    return np.concatenate(parts), un


def _gamma():
    """x_sb[p, t] holds x'[gamma[p, t]].  Chunk c (t in [16c,16c+16)) covers
    each core's y[256c:256c+256] in AllGather-output flat order."""
    p = np.arange(P)[:, None]
    t = np.arange(NT)[None, :]
    return (p // 16) * 1024 + (t // CT) * CW + (p % 16) * CT + (t % CT)


def _build_nc(n_iter, clamp_lo, l_val, u_val):
    import concourse.bacc as bacc
    import concourse.mybir as mybir
    from concourse import tile
    from concourse.bass import _add_dep_helper

    # clamp region must sit inside the last chunk
    cl = clamp_lo - (NCHUNKS - 1) * CW
    assert 0 <= cl < CW
    nc = bacc.Bacc(None, target_bir_lowering=False, num_devices=NCORES)
    NWCH = 16  # W arrives as 16 chunks of 4 k-tiles each
    w_ext = [
        nc.declare_dram_parameter(
            f"W{c}", [P, (NT // NWCH) * ROWS], mybir.dt.bfloat16, isOutput=False
        )
        for c in range(NWCH)
    ]
    x0_ext = nc.declare_dram_parameter("x0", [P, NT], mybir.dt.bfloat16, isOutput=False)
    b_ext = nc.declare_dram_parameter("bias", [1, ROWS], mybir.dt.float32, isOutput=False)
    out_ext = nc.declare_dram_parameter("out", [1, ROWS], mybir.dt.float32, isOutput=True)

    with tile.TileContext(nc) as tc:
        with (
            tc.tile_pool(name="wpool", bufs=1) as wpool,
            tc.tile_pool(name="cpool", bufs=1) as cpool,
            tc.tile_pool(name="xpool", bufs=2) as xpool,
            tc.tile_pool(name="ypool", bufs=2) as ypool,
            tc.tile_pool(name="ps", bufs=2, space="PSUM") as pspool,
            tc.tile_pool(name="dram", bufs=2, space="DRAM") as dpool,
        ):
            wt = []
            for c in range(NWCH):
                w = wpool.tile([P, (NT // NWCH) * ROWS], mybir.dt.bfloat16, tag=f"W{c}")
                nc.sync.dma_start(w[:], w_ext[c][:])
                wt.append(w)
            b_sb = cpool.tile([1, ROWS], mybir.dt.float32, tag="b")
            nc.sync.dma_start(b_sb[:], b_ext[:])

            xb = []
            for c in range(NCHUNKS):
                x = xpool.tile([P, CT], mybir.dt.bfloat16, tag=f"xb{c}")
                nc.sync.dma_start(x[:], x0_ext[:, c * CT : (c + 1) * CT])
                xb.append(x)

            # Hybrid schedule: for the first PAIR k-tiles, psA's and psB's
            # matmuls are paired on the same x-tile (the second MM shares the
            # LDWEIGHTS window of the first -> ~432ns/pair instead of
            # 2x259ns).  The remaining tiles run solo-psA then solo-psB so
            # chunk A still completes early and its AllGather hides under
            # the rest of the iteration.  prev_mm chains every MM to keep
            # the Tile scheduler from reordering the PE stream.
            prev_mm = [None]

            def mm(ps, c, t, start, stop):
                wc, r = t // 4, t % 4
                m = nc.tensor.matmul(
                    ps[:, :],
                    xb[t // CT][:, t % CT : t % CT + 1],
                    wt[wc][:, r * ROWS + c * CW : r * ROWS + (c + 1) * CW],
                    start=start,
                    stop=stop,
                )
                if prev_mm[0] is not None:
                    _add_dep_helper(
                        m.ins, prev_mm[0].ins, sync=False, reason="PE order"
                    )
                prev_mm[0] = m

            def epilogue(ps, c, last):
                is_clamp = c == NCHUNKS - 1
                ydt = mybir.dt.float32 if last else mybir.dt.bfloat16
                y = ypool.tile([1, CW], ydt, tag=f"y{c}_{'f' if last else 'b'}")
                nc.vector.tensor_tensor(
                    y[:, :],
                    ps[:, :],
                    b_sb[:, c * CW : (c + 1) * CW],
                    op=mybir.AluOpType.add,
                )
                if is_clamp:
                    nc.vector.tensor_scalar(
                        y[:, cl:CW],
                        y[:, cl:CW],
                        float(l_val),
                        float(u_val),
                        mybir.AluOpType.max,
                        mybir.AluOpType.min,
                    )
                if last:
                    nc.sync.dma_start(out_ext[:, c * CW : (c + 1) * CW], y[:])
                    return None
                agin = dpool.tile([1, CW], mybir.dt.bfloat16, tag=f"agin{c}")
                agout = dpool.tile([P, CT], mybir.dt.bfloat16, tag=f"agout{c}")
                nc.gpsimd.dma_start(agin[:], y[:])
                nc.gpsimd.collective_compute(
                    "AllGather",
                    mybir.AluOpType.bypass,
                    replica_groups=[list(range(NCORES))],
                    ins=[agin.opt()],
                    outs=[agout.opt()],
                )
                xn = xpool.tile([P, CT], mybir.dt.bfloat16, tag=f"xb{c}")
                nc.scalar.dma_start(xn[:], agout[:])
                return xn

            for k in range(n_iter):
                last = k == n_iter - 1
                xb_next = list(xb)
                psA = pspool.tile([1, CW], mybir.dt.float32, tag="ps0")
                psB = pspool.tile([1, CW], mybir.dt.float32, tag="ps1")
                for t in range(PAIR):
                    mm(psA, 0, t, start=(t == 0), stop=False)
                    mm(psB, 1, t, start=(t == 0), stop=False)
                for t in range(PAIR, NT):
                    mm(psA, 0, t, start=False, stop=(t == NT - 1))
                xn = epilogue(psA, 0, last)
                if xn is not None:
                    xb_next[0] = xn
                for t in range(PAIR, NT):
                    mm(psB, 1, t, start=False, stop=(t == NT - 1))
                xn = epilogue(psB, 1, last)
                if xn is not None:
                    xb_next[1] = xn
                xb = xb_next
    nc.compile()
    return nc


def _get_nc(n_iter, clamp_lo, l_val, u_val):
    key = (n_iter, clamp_lo, float(l_val), float(u_val))
    if key not in _nc_cache:
        _nc_cache[key] = _build_nc(n_iter, clamp_lo, l_val, u_val)
    return _nc_cache[key]


def _prep_in_maps(x, W, b, idx1, idx2):
    perm, _un = _perm(idx1, idx2)
    g = _gamma()
    colidx = perm[g]  # [128, 64] original column index per (p, t)
    xp = np.asarray(x, np.float32)[perm]
    bp = np.asarray(b, np.float32)[perm]
    bf16 = ml_dtypes.bfloat16
    x0_layout = np.ascontiguousarray(xp[g]).astype(bf16)
    NWCH = 16
    in_maps = []
    for i in range(NCORES):
        rows_i = perm[ROWS * i : ROWS * (i + 1)]
        Wi = W[rows_i]  # [1024, 8192]
        Wc = Wi[:, colidx.reshape(-1)].reshape(ROWS, P, NT)  # [n, p, t]
        Wt = np.ascontiguousarray(
            np.transpose(Wc, (1, 2, 0)).reshape(P, NT * ROWS)
        ).astype(bf16)  # Wt[p, t*1024 + n]
        m = {
            f"W{c}": np.ascontiguousarray(
                Wt[:, c * (NT // NWCH) * ROWS : (c + 1) * (NT // NWCH) * ROWS]
            )
            for c in range(NWCH)
        }
        m["x0"] = x0_layout
        m["bias"] = np.ascontiguousarray(bp[ROWS * i : ROWS * (i + 1)].reshape(1, ROWS))
        in_maps.append(m)
    return in_maps, perm


def run(x, W, b, l, u, idx1, idx2, N, trace=False, trace_kwargs=None):
    from concourse.bass_utils import run_bass_kernel_spmd

    x = np.asarray(x, np.float32)
    W = np.asarray(W, np.float32)
    b = np.asarray(b, np.float32)
    l = float(np.asarray(l))
    u = float(np.asarray(u))
    idx1 = int(np.asarray(idx1))
    idx2 = int(np.asarray(idx2))
    N = int(np.asarray(N))
    assert x.shape == (D,) and W.shape == (D, D) and b.shape == (D,)
    assert N >= 1

    seg = (idx2 - idx1) // NCORES
    clamp_lo = ROWS - seg
    nc = _get_nc(N, clamp_lo, l, u)
    in_maps, perm = _prep_in_maps(x, W, b, idx1, idx2)
    res = run_bass_kernel_spmd(
        nc,
        in_maps,
        core_ids=list(range(NCORES)),
        trace=trace,
        **(trace_kwargs or {}),
    )
    chunks = [np.asarray(res.results[i]["out"], np.float32).reshape(ROWS) for i in range(NCORES)]
    xp_final = np.concatenate(chunks)
    out = np.empty(D, np.float32)
    out[perm] = xp_final
    return out, res


def kernel(**inputs):
    out, _ = run(
        inputs["x"],
        inputs["W"],
        inputs["b"],
        inputs["l"],
        inputs["u"],
        inputs["idx1"],
        inputs["idx2"],
        inputs["N"],
        trace=False,
    )
    return out
